# Initial kernel scaffold
#
"""TRN2 Bass kernel for nn_CosClassifier: sim = 10*scalar * cos_sim(inputs, proto).

Data-parallel over 8 NeuronCores: each core computes a (2048, 4096) slab of the
(16384, 4096) similarity matrix. Per core:
  1. DMA in x-slab (2048,256) in 4x512KB subgroups, proto (4096,256) in
     8x512KB subgroups, interleaved so compute starts early.
  2. Per subgroup as it lands: row norms (ACT Square+accum -> Sqrt, DVE
     reciprocal), row scaling (x by 10/||x||, proto by scalar/||p||), then
     PE-transposes 128x128-blockwise, 4 per PSUM bank; one 512-wide
     PSUM->SBUF copy per bank casts to float32r (TF32-like) so the main
     matmul runs at 1 cycle/row.
  3. dots matmul in fp32r (k-alternating lhsT; same-lhsT b2b fp32r is
     pathologically slow), fp32 accumulate in PSUM; plain-copy drains split
     ACT/DVE in four phases of n-block pairs so 512KB contiguous output DMAs
     flow from ~20us.
"""
import sys

sys.path.insert(0, "/opt/trn_rl_repo")

import numpy as np

B, C, D = 16384, 4096, 256
NCORES = 8
BS = B // NCORES          # 2048 rows per core
NB = BS // 128            # 16 b-tiles per core
NCT = C // 128            # 32 c-tiles (proto rows)
NK = D // 128             # 2 k-tiles
NN = C // 512             # 8 n-blocks of 512
SGT = 4                   # tiles per subgroup (512KB)
XSG = NB // SGT           # 4 x subgroups
PSG = NCT // SGT          # 8 proto subgroups

_compiled = None


def _build():
    import concourse.bacc as bacc
    import concourse.mybir as mybir
    import concourse.tile as tile

    f32 = mybir.dt.float32
    f32r = mybir.dt.float32r
    Act = mybir.ActivationFunctionType

    nc = bacc.Bacc("TRN2", target_bir_lowering=False, debug=False,
                   num_devices=NCORES)

    x_d = nc.dram_tensor("x", [BS, D], f32, kind="ExternalInput").ap()
    p_d = nc.dram_tensor("proto", [C, D], f32, kind="ExternalInput").ap()
    s_d = nc.dram_tensor("scalar", [1, 1], f32, kind="ExternalInput").ap()
    id_d = nc.dram_tensor("identity", [128, 128], f32, kind="ExternalInput").ap()
    out_d = nc.dram_tensor("out", [BS, C], f32, kind="ExternalOutput").ap()

    with tile.TileContext(nc) as tc:
        with tc.tile_pool(name="sbuf", bufs=1) as pool, \
             tc.tile_pool(name="outp", bufs=6) as outp, \
             tc.tile_pool(name="psum_t", bufs=2, space="PSUM") as psum_t, \
             tc.tile_pool(name="psum_m", bufs=6, space="PSUM") as psum_m:

            x_r = x_d.rearrange("(n p) d -> p n d", p=128)       # [128, NB, 256]
            p_r = p_d.rearrange("(n p) d -> p n d", p=128)       # [128, NCT, 256]

            def load_x(g):
                t = pool.tile([128, SGT * D], f32, tag=f"xsg{g}")
                nc.sync.dma_start(
                    t[:].rearrange("p (n d) -> p n d", d=D),
                    x_r[:, g * SGT:(g + 1) * SGT, :])
                return t

            def load_p(g):
                t = pool.tile([128, SGT * D], f32, tag=f"psg{g}")
                nc.sync.dma_start(
                    t[:].rearrange("p (n d) -> p n d", d=D),
                    p_r[:, g * SGT:(g + 1) * SGT, :])
                return t

            xsg = {}
            psg = {}
            xsg[0] = load_x(0)
            ident = pool.tile([128, 128], f32, tag="ident")
            nc.sync.dma_start(ident[:], id_d[:, :])
            sc = pool.tile([1, 1], f32, tag="sc")
            nc.sync.dma_start(sc[:], s_d[:, :])
            sc_b = pool.tile([128, 1], f32, tag="sc_b")
            nc.gpsimd.partition_broadcast(sc_b[:], sc[:])
            psg[0] = load_p(0)
            psg[1] = load_p(1)
            xsg[1] = load_x(1)
            psg[2] = load_p(2)
            xsg[2] = load_x(2)
            psg[3] = load_p(3)
            xsg[3] = load_x(3)
            for g in range(4, PSG):
                psg[g] = load_p(g)

            # transposed operands (f32r)
            # xt: k-block k at cols k*BS, b-tile i at +i*128
            xt = pool.tile([128, NK * BS], f32r, tag="xt")
            # pt: k-block k at cols k*C, c-tile j at +j*128
            pt = pool.tile([128, NK * C], f32r, tag="pt")

            cast_flip = [0]

            def process_subgroup(grp, gi, with_scalar, dst, dst_stride):
                for t in range(SGT):
                    src = grp[:, t * D:(t + 1) * D]
                    ssq = pool.tile([128, 1], f32, tag=f"ssq{t % 2}")
                    sq_scr = pool.tile([128, D], f32, tag=f"sqscr{t % 2}")
                    nc.scalar.activation(sq_scr[:], src, Act.Square,
                                         accum_out=ssq[:])
                    nrm = pool.tile([128, 1], f32, tag=f"nrm{t % 2}")
                    # x: sqrt(ssq)/10 (folds *10); proto: plain norm
                    nc.scalar.activation(nrm[:], ssq[:], Act.Sqrt,
                                         scale=1.0 if with_scalar else 0.01)
                    inv = pool.tile([128, 1], f32, tag=f"inv{t % 2}")
                    nc.vector.reciprocal(inv[:], nrm[:])
                    if with_scalar:
                        nc.vector.tensor_mul(inv[:], inv[:], sc_b[:])
                    nc.vector.tensor_scalar_mul(src, src, inv[:])
                # 4 transposes share one PSUM bank; one 512-wide cast drains it
                for k in range(NK):
                    tp = psum_t.tile([128, SGT * 128], f32, tag="tp")
                    for t in range(SGT):
                        nc.tensor.transpose(
                            tp[:, t * 128:(t + 1) * 128],
                            grp[:, t * D + k * 128: t * D + (k + 1) * 128],
                            ident[:])
                    cdst = dst[:, k * dst_stride + gi * SGT * 128:
                               k * dst_stride + (gi + 1) * SGT * 128]
                    # casts alternate ACT/DVE
                    if cast_flip[0] % 2 == 0:
                        nc.scalar.copy(cdst, tp[:])
                    else:
                        nc.vector.tensor_copy(cdst, tp[:])
                    cast_flip[0] += 1

            # process in DMA arrival order, x/p interleaved
            process_subgroup(xsg[0], 0, False, xt, BS)
            process_subgroup(psg[0], 0, True, pt, C)
            process_subgroup(psg[1], 1, True, pt, C)
            process_subgroup(xsg[1], 1, False, xt, BS)
            process_subgroup(psg[2], 2, True, pt, C)
            process_subgroup(xsg[2], 2, False, xt, BS)
            process_subgroup(psg[3], 3, True, pt, C)
            process_subgroup(xsg[3], 3, False, xt, BS)
            for g in range(4, PSG):
                process_subgroup(psg[g], g, True, pt, C)

            # ---- main matmul + drain ----
            # phase h covers n-blocks {2h, 2h+1} <-> proto subgroups 2h,2h+1,
            # so MMs start as soon as the matching proto subgroup is ready and
            # 512KB contiguous output DMAs flow from early in the kernel.
            drain_flip = [0]
            for h in range(NN // 2):
                for i in range(NB):
                    oq = outp.tile([128, 1024], f32, tag="oq")
                    for nn_ in range(2):
                        n = 2 * h + nn_
                        ps = psum_m.tile([128, 512], f32, tag="mm")
                        for k in range(NK):
                            nc.tensor.matmul(
                                ps[:],
                                xt[:, k * BS + i * 128: k * BS + (i + 1) * 128],
                                pt[:, k * C + n * 512: k * C + (n + 1) * 512],
                                start=(k == 0), stop=(k == NK - 1))
                        dst = oq[:, nn_ * 512:(nn_ + 1) * 512]
                        # drains: 5 of 16 on ACT, 11 of 16 on DVE
                        if (drain_flip[0] * 4) % 16 < 4:
                            nc.scalar.copy(dst, ps[:])
                        else:
                            nc.vector.tensor_copy(dst, ps[:])
                        drain_flip[0] += 1
                    nc.sync.dma_start(
                        out_d[i * 128:(i + 1) * 128,
                              h * 1024:(h + 1) * 1024], oq[:])

    nc.compile()
    return nc


def _get_compiled():
    global _compiled
    if _compiled is None:
        _compiled = _build()
    return _compiled


def kernel(inputs, proto, scalar, _trace=False, **_tr_kw):
    from concourse.bass_utils import run_bass_kernel_spmd

    nc = _get_compiled()
    inputs = np.ascontiguousarray(inputs, dtype=np.float32)
    proto = np.ascontiguousarray(proto, dtype=np.float32)
    sc = np.asarray(scalar, dtype=np.float32).reshape(1, 1)
    ident = np.eye(128, dtype=np.float32)

    in_maps = []
    for c in range(NCORES):
        in_maps.append({
            "x": inputs[c * BS:(c + 1) * BS],
            "proto": proto,
            "scalar": sc,
            "identity": ident,
        })
    res = run_bass_kernel_spmd(nc, in_maps, core_ids=list(range(NCORES)),
                               trace=_trace, **_tr_kw)
    out = np.concatenate([res.results[c]["out"] for c in range(NCORES)], axis=0)
    if _trace:
        kernel.last_results = res
    return out



# revision 12
# speedup vs baseline: 1.1411x; 1.1411x over previous
"""TRN2 Bass kernel for nn_CosClassifier: sim = 10*scalar * cos_sim(inputs, proto).

Data-parallel over 8 NeuronCores: each core computes a (2048, 4096) slab of the
(16384, 4096) similarity matrix. The kernel is HBM-bound (32MB out + 6MB in per
core ~ 106us at ~358GB/s), so everything is organized to keep the output DMA
stream saturated right as the input stream ends (~26us):
  1. identity/scalar are DMA'd from the scalar (ACT HWDGE) queue so they land
     in parallel with the first input chunks on the sync queue; input order
     x0,p0,p1,x1,p2,p3,... matches what each output phase needs first.
  2. x is NOT pre-scaled: its 10/||x|| factor is folded into the PSUM->SBUF
     output drains (ACT activation-Copy-with-scale / DVE tensor_scalar_mul),
     which cost the same as plain copies. Only proto rows are pre-scaled by
     scalar/||p||. Norms via ACT Square+accum (the serial ACT chain paces the
     front); sqrt/reciprocal batched [128,4] per subgroup; the first six
     subgroup casts go to DVE so they never head-of-line block the ACT chain.
  3. Operands are PE-transposed 128x128 blockwise and cast fp32->fp16 by one
     wide PSUM->SBUF copy per (subgroup, k); fp16 keeps LDWEIGHTS fast (FWL).
  4. Main GEMM in four 1024-wide column phases (phase h needs only proto
     subgroups 2h,2h+1); per (phase, b-tile): 4 fp16 matmuls (k-outer) into
     one 2-bank PSUM tile, one 1024-wide scaling drain, one 512KB output DMA.
     Emission is interleaved with late x/proto processing so engine FIFOs
     never head-of-line block early drains.
"""
import sys

sys.path.insert(0, "/opt/trn_rl_repo")

import numpy as np

B, C, D = 16384, 4096, 256
NCORES = 8
BS = B // NCORES          # 2048 rows per core
NB = BS // 128            # 16 b-tiles per core
NCT = C // 128            # 32 c-tiles (proto rows)
NK = D // 128             # 2 k-tiles
NN = C // 512             # 8 n-blocks of 512
SGT = 4                   # tiles per subgroup (512KB)
XSG = NB // SGT           # 4 x subgroups
PSG = NCT // SGT          # 8 proto subgroups
NPH = 4                   # output column phases (1024 wide each)

_compiled = None


def _build():
    import concourse.bacc as bacc
    import concourse.mybir as mybir
    import concourse.tile as tile

    f32 = mybir.dt.float32
    f16 = mybir.dt.float16
    Act = mybir.ActivationFunctionType

    nc = bacc.Bacc("TRN2", target_bir_lowering=False, debug=False,
                   num_devices=NCORES)

    x_d = nc.dram_tensor("x", [BS, D], f32, kind="ExternalInput").ap()
    p_d = nc.dram_tensor("proto", [C, D], f32, kind="ExternalInput").ap()
    s_d = nc.dram_tensor("scalar", [1, 1], f32, kind="ExternalInput").ap()
    id_d = nc.dram_tensor("identity", [128, 128], f32, kind="ExternalInput").ap()
    out_d = nc.dram_tensor("out", [BS, C], f32, kind="ExternalOutput").ap()

    with tile.TileContext(nc) as tc:
        with tc.tile_pool(name="sbuf", bufs=1) as pool, \
             tc.tile_pool(name="outp", bufs=10) as outp, \
             tc.tile_pool(name="psum_t", bufs=2, space="PSUM") as psum_t, \
             tc.tile_pool(name="psum_m", bufs=3, space="PSUM") as psum_m:

            x_r = x_d.rearrange("(n p) d -> p n d", p=128)       # [128, NB, 256]
            p_r = p_d.rearrange("(n p) d -> p n d", p=128)       # [128, NCT, 256]

            def load_x(g):
                t = pool.tile([128, SGT * D], f32, tag=f"xsg{g}")
                nc.sync.dma_start(
                    t[:].rearrange("p (n d) -> p n d", d=D),
                    x_r[:, g * SGT:(g + 1) * SGT, :])
                return t

            def load_p(g):
                t = pool.tile([128, SGT * D], f32, tag=f"psg{g}")
                nc.sync.dma_start(
                    t[:].rearrange("p (n d) -> p n d", d=D),
                    p_r[:, g * SGT:(g + 1) * SGT, :])
                return t

            # identity + scalar via the ACT HWDGE queue: they land in
            # parallel with the first big input chunks on the sync queue
            ident = pool.tile([128, 128], f32, tag="ident")
            nc.scalar.dma_start(ident[:], id_d[:, :])
            sc = pool.tile([1, 1], f32, tag="sc")
            nc.scalar.dma_start(sc[:], s_d[:, :])
            sc_b = pool.tile([128, 1], f32, tag="sc_b")
            nc.gpsimd.partition_broadcast(sc_b[:], sc[:])

            xsg = {}
            psg = {}
            xsg[0] = load_x(0)
            psg[0] = load_p(0)
            psg[1] = load_p(1)
            xsg[1] = load_x(1)
            psg[2] = load_p(2)
            psg[3] = load_p(3)
            xsg[2] = load_x(2)
            xsg[3] = load_x(3)
            for g in range(PSG // 2, PSG):
                psg[g] = load_p(g)

            # transposed fp16 operands, tile-major with k interleaved:
            # xt: b-tile i at cols i*256, k-block k at +k*128 (x is UNSCALED)
            xt = pool.tile([128, NB * D], f16, tag="xt")
            # pt: c-tile j at cols j*256, k-block k at +k*128 (rows scaled)
            pt = pool.tile([128, NCT * D], f16, tag="pt")
            xt_r = xt[:].rearrange("p (i two d) -> p i two d", two=NK, d=128)
            pt_r = pt[:].rearrange("p (j two d) -> p j two d", two=NK, d=128)
            # 10/||x_b|| per b-tile, used to scale output drains
            xinv = pool.tile([128, NB], f32, tag="xinv")

            # each subgroup's two casts are split ACT(k0)/DVE(k1): halves the
            # per-subgroup cast latency in the processing chain and spreads
            # the load
            drain_flip = [0]

            def transpose_cast(grp, gi, dst_r):
                # 4 transposes share one PSUM bank; one 512-wide fp16 cast
                # per k drains it (strided dst: 4 chunks at stride 256)
                for k in range(NK):
                    tp = psum_t.tile([128, SGT * 128], f32, tag="tp")
                    for t in range(SGT):
                        nc.tensor.transpose(
                            tp[:, t * 128:(t + 1) * 128],
                            grp[:, t * D + k * 128: t * D + (k + 1) * 128],
                            ident[:])
                    cdst = dst_r[:, gi * SGT:(gi + 1) * SGT, k, :]
                    if k == 0:
                        nc.scalar.copy(cdst, tp[:])
                    else:
                        nc.vector.tensor_copy(cdst, tp[:])

            def norms4(grp, tag):
                # sum-of-squares per row for the 4 tiles of a subgroup,
                # batched into [128, 4]
                ssq4 = pool.tile([128, SGT], f32, tag=f"ssq4{tag}")
                sq_scr = pool.tile([128, D], f32, tag=f"sqscr{tag}")
                for t in range(SGT):
                    nc.scalar.activation(sq_scr[:], grp[:, t * D:(t + 1) * D],
                                         Act.Square, accum_out=ssq4[:, t:t + 1])
                return ssq4

            def px_cast(g):
                # transpose/cast has no scaling dependency for x (unscaled)
                transpose_cast(xsg[g], g, xt_r)

            def px_norms(g):
                ssq4 = norms4(xsg[g], "x")
                nrm4 = pool.tile([128, SGT], f32, tag="nrm4x")
                # sqrt(0.01*ssq) = ||x||/10; reciprocal -> 10/||x||
                nc.scalar.activation(nrm4[:], ssq4[:], Act.Sqrt, scale=0.01)
                nc.vector.reciprocal(xinv[:, g * SGT:(g + 1) * SGT], nrm4[:])

            # proto processing is staged (norms / scale / transpose+cast
            # emitted as separate waves across subgroups) so the per-subgroup
            # ACT->DVE->PE->cast chains pipeline instead of serializing in
            # the engine FIFOs.
            pinv = {}

            def p_norms(g):
                ssq4 = norms4(psg[g], "p")
                nrm4 = pool.tile([128, SGT], f32, tag=f"nrm4p{g % 2}")
                nc.scalar.activation(nrm4[:], ssq4[:], Act.Sqrt)
                inv4 = pool.tile([128, SGT], f32, tag=f"inv4p{g}")
                nc.vector.reciprocal(inv4[:], nrm4[:])
                nc.vector.tensor_scalar_mul(inv4[:], inv4[:], sc_b[:])
                pinv[g] = inv4

            def p_scale(g):
                for t in range(SGT):
                    src = psg[g][:, t * D:(t + 1) * D]
                    nc.vector.tensor_scalar_mul(src, src, pinv[g][:, t:t + 1])

            def p_transcast(g):
                transpose_cast(psg[g], g, pt_r)

            def p_pair(g0):
                # pipelined emission over subgroups g0, g0+1
                p_norms(g0)
                p_norms(g0 + 1)
                p_scale(g0)
                p_transcast(g0)
                p_scale(g0 + 1)
                p_transcast(g0 + 1)

            # ---- main matmul + scaling drain ----
            # phase h covers n-blocks {2h, 2h+1} (proto subgroups 2h, 2h+1);
            # per b-tile i: 4 fp16 MMs (k-outer) into one 2-bank PSUM tile,
            # one 1024-wide drain that also applies 10/||x_b||, one 512KB
            # output DMA (128 rows x 4KB).
            def mm(h, i):
                oq = outp.tile([128, 1024], f32, tag="oq")
                ps = psum_m.tile([128, 1024], f32, tag="mm")
                for k in range(NK):
                    for nn_ in range(2):
                        n = 2 * h + nn_
                        nc.tensor.matmul(
                            ps[:, nn_ * 512:(nn_ + 1) * 512],
                            xt_r[:, i, k, :],
                            pt_r[:, 4 * n:4 * n + 4, k, :],
                            start=(k == 0), stop=(k == NK - 1))
                inv = xinv[:, i:i + 1]
                if drain_flip[0] % 16 < 6:
                    nc.scalar.activation(oq[:], ps[:], Act.Copy, scale=inv)
                else:
                    nc.vector.tensor_scalar_mul(oq[:], ps[:], inv)
                drain_flip[0] += 1
                nc.sync.dma_start(
                    out_d[i * 128:(i + 1) * 128,
                          h * 1024:(h + 1) * 1024], oq[:])

            # emission interleaved with processing so engine FIFOs don't
            # head-of-line block early drains, and late processing is spread
            # thin across the mm stream so it never stalls the output DMAs.
            # Constraints: px_cast(g)/px_norms(g) before mm(*, 4g);
            # p_scale/p_transcast(2h..2h+1) before mm(h, 0).
            process_sched = {
                0: {1: [lambda: px_cast(1)],
                    3: [lambda: px_norms(1)],
                    4: [lambda: p_norms(2)],
                    5: [lambda: px_cast(2)],
                    6: [lambda: px_norms(2)],
                    7: [lambda: p_norms(3)],
                    8: [lambda: p_scale(2)],
                    9: [lambda: px_cast(3)],
                    10: [lambda: p_transcast(2)],
                    11: [lambda: px_norms(3)],
                    12: [lambda: p_scale(3)],
                    13: [lambda: p_transcast(3)]},
                1: {1: [lambda: p_norms(4)],
                    3: [lambda: p_norms(5)],
                    5: [lambda: p_scale(4)],
                    7: [lambda: p_transcast(4)],
                    9: [lambda: p_scale(5)],
                    11: [lambda: p_transcast(5)]},
                2: {1: [lambda: p_norms(6)],
                    3: [lambda: p_norms(7)],
                    5: [lambda: p_scale(6)],
                    7: [lambda: p_transcast(6)],
                    9: [lambda: p_scale(7)],
                    11: [lambda: p_transcast(7)]},
                3: {},
            }
            px_cast(0)
            px_norms(0)
            p_pair(0)
            for h in range(NPH):
                for i in range(NB):
                    mm(h, i)
                    for fn in process_sched[h].get(i, []):
                        fn()

    nc.compile()
    return nc


def _get_compiled():
    global _compiled
    if _compiled is None:
        _compiled = _build()
    return _compiled


def kernel(inputs, proto, scalar, _trace=False, **_tr_kw):
    from concourse.bass_utils import run_bass_kernel_spmd

    nc = _get_compiled()
    inputs = np.ascontiguousarray(inputs, dtype=np.float32)
    proto = np.ascontiguousarray(proto, dtype=np.float32)
    sc = np.asarray(scalar, dtype=np.float32).reshape(1, 1)
    ident = np.eye(128, dtype=np.float32)

    in_maps = []
    for c in range(NCORES):
        in_maps.append({
            "x": inputs[c * BS:(c + 1) * BS],
            "proto": proto,
            "scalar": sc,
            "identity": ident,
        })
    res = run_bass_kernel_spmd(nc, in_maps, core_ids=list(range(NCORES)),
                               trace=_trace, **_tr_kw)
    out = np.concatenate([res.results[c]["out"] for c in range(NCORES)], axis=0)
    if _trace:
        kernel.last_results = res
    return out


# revision 17
# speedup vs baseline: 1.1820x; 1.0358x over previous
"""TRN2 Bass kernel for nn_CosClassifier: sim = 10*scalar * cos_sim(inputs, proto).

Data-parallel over 8 NeuronCores: each core computes a (2048, 4096) slab of the
(16384, 4096) similarity matrix. The kernel is HBM-bound (32MB out + 6MB in per
core ~ 106us at ~358GB/s), so everything is organized to keep the output DMA
stream saturated right as the input stream ends (~26us):
  1. identity/scalar are DMA'd from the scalar (ACT HWDGE) queue so they land
     in parallel with the first input chunks on the sync queue; input order
     x0,p0,p1,x1,p2,p3,... matches what each output phase needs first.
  2. x is NOT pre-scaled: its 10/||x|| factor is folded into the PSUM->SBUF
     output drains (ACT activation-Copy-with-scale / DVE tensor_scalar_mul),
     which cost the same as plain copies. Only proto rows are pre-scaled by
     scalar/||p||. Norms via ACT Square+accum (the serial ACT chain paces the
     front); sqrt/reciprocal batched [128,4] per subgroup; the first six
     subgroup casts go to DVE so they never head-of-line block the ACT chain.
  3. Operands are PE-transposed 128x128 blockwise and cast fp32->fp16 by one
     wide PSUM->SBUF copy per (subgroup, k); fp16 keeps LDWEIGHTS fast (FWL).
  4. Main GEMM in four 1024-wide column phases (phase h needs only proto
     subgroups 2h,2h+1); per (phase, b-tile): 4 fp16 matmuls (k-outer) into
     one 2-bank PSUM tile, one 1024-wide scaling drain, one 512KB output DMA.
     Emission is interleaved with late x/proto processing so engine FIFOs
     never head-of-line block early drains.
"""
import sys

sys.path.insert(0, "/opt/trn_rl_repo")

import numpy as np

B, C, D = 16384, 4096, 256
NCORES = 8
BS = B // NCORES          # 2048 rows per core
NB = BS // 128            # 16 b-tiles per core
NCT = C // 128            # 32 c-tiles (proto rows)
NK = D // 128             # 2 k-tiles
NN = C // 512             # 8 n-blocks of 512
SGT = 4                   # tiles per subgroup (512KB)
XSG = NB // SGT           # 4 x subgroups
PSG = NCT // SGT          # 8 proto subgroups
NPH = 4                   # output column phases (1024 wide each)

_compiled = None


def _build():
    import concourse.bacc as bacc
    import concourse.mybir as mybir
    import concourse.tile as tile

    f32 = mybir.dt.float32
    f16 = mybir.dt.float16
    Act = mybir.ActivationFunctionType

    nc = bacc.Bacc("TRN2", target_bir_lowering=False, debug=False,
                   num_devices=NCORES)

    x_d = nc.dram_tensor("x", [BS, D], f32, kind="ExternalInput").ap()
    p_d = nc.dram_tensor("proto", [C, D], f32, kind="ExternalInput").ap()
    s_d = nc.dram_tensor("scalar", [1, 1], f32, kind="ExternalInput").ap()
    id_d = nc.dram_tensor("identity", [128, 128], f32, kind="ExternalInput").ap()
    out_d = nc.dram_tensor("out", [BS, C], f32, kind="ExternalOutput").ap()

    with tile.TileContext(nc) as tc:
        with tc.tile_pool(name="sbuf", bufs=1) as pool, \
             tc.tile_pool(name="outp", bufs=10) as outp, \
             tc.tile_pool(name="psum_t", bufs=2, space="PSUM") as psum_t, \
             tc.tile_pool(name="psum_m", bufs=3, space="PSUM") as psum_m:

            x_r = x_d.rearrange("(n p) d -> p n d", p=128)       # [128, NB, 256]
            p_r = p_d.rearrange("(n p) d -> p n d", p=128)       # [128, NCT, 256]

            def load_x(g):
                t = pool.tile([128, SGT * D], f32, tag=f"xsg{g}")
                nc.sync.dma_start(
                    t[:].rearrange("p (n d) -> p n d", d=D),
                    x_r[:, g * SGT:(g + 1) * SGT, :])
                return t

            def load_p(g):
                t = pool.tile([128, SGT * D], f32, tag=f"psg{g}")
                nc.sync.dma_start(
                    t[:].rearrange("p (n d) -> p n d", d=D),
                    p_r[:, g * SGT:(g + 1) * SGT, :])
                return t

            # identity + scalar via the ACT HWDGE queue: they land in
            # parallel with the first big input chunks on the sync queue
            ident = pool.tile([128, 128], f32, tag="ident")
            nc.scalar.dma_start(ident[:], id_d[:, :])
            sc = pool.tile([1, 1], f32, tag="sc")
            nc.scalar.dma_start(sc[:], s_d[:, :])
            sc_b = pool.tile([128, 1], f32, tag="sc_b")
            nc.gpsimd.partition_broadcast(sc_b[:], sc[:])

            xsg = {}
            psg = {}
            xsg[0] = load_x(0)
            psg[0] = load_p(0)
            psg[1] = load_p(1)
            xsg[1] = load_x(1)
            psg[2] = load_p(2)
            psg[3] = load_p(3)
            xsg[2] = load_x(2)
            xsg[3] = load_x(3)
            for g in range(PSG // 2, PSG):
                psg[g] = load_p(g)

            # transposed fp16 operands, tile-major with k interleaved:
            # xt: b-tile i at cols i*256, k-block k at +k*128 (x is UNSCALED)
            xt = pool.tile([128, NB * D], f16, tag="xt")
            # pt: c-tile j at cols j*256, k-block k at +k*128 (rows scaled)
            pt = pool.tile([128, NCT * D], f16, tag="pt")
            xt_r = xt[:].rearrange("p (i two d) -> p i two d", two=NK, d=128)
            pt_r = pt[:].rearrange("p (j two d) -> p j two d", two=NK, d=128)
            # 10/||x_b|| per b-tile, used to scale output drains
            xinv = pool.tile([128, NB], f32, tag="xinv")

            # each subgroup's two casts are split ACT(k0)/DVE(k1): halves the
            # per-subgroup cast latency in the processing chain and spreads
            # the load
            def transpose_cast(grp, gi, dst_r):
                # 4 transposes share one PSUM bank; one 512-wide fp16 cast
                # per k drains it (strided dst: 4 chunks at stride 256)
                for k in range(NK):
                    tp = psum_t.tile([128, SGT * 128], f32, tag="tp")
                    for t in range(SGT):
                        nc.tensor.transpose(
                            tp[:, t * 128:(t + 1) * 128],
                            grp[:, t * D + k * 128: t * D + (k + 1) * 128],
                            ident[:])
                    cdst = dst_r[:, gi * SGT:(gi + 1) * SGT, k, :]
                    if k == 0:
                        nc.scalar.copy(cdst, tp[:])
                    else:
                        nc.vector.tensor_copy(cdst, tp[:])

            def norms4(grp, tag):
                # sum-of-squares per row for the 4 tiles of a subgroup,
                # batched into [128, 4]
                ssq4 = pool.tile([128, SGT], f32, tag=f"ssq4{tag}")
                sq_scr = pool.tile([128, D], f32, tag=f"sqscr{tag}")
                for t in range(SGT):
                    nc.scalar.activation(sq_scr[:], grp[:, t * D:(t + 1) * D],
                                         Act.Square, accum_out=ssq4[:, t:t + 1])
                return ssq4

            def px_cast(g):
                # transpose/cast has no scaling dependency for x (unscaled)
                transpose_cast(xsg[g], g, xt_r)

            def px_norms(g):
                ssq4 = norms4(xsg[g], "x")
                nrm4 = pool.tile([128, SGT], f32, tag="nrm4x")
                # sqrt(0.01*ssq) = ||x||/10; reciprocal -> 10/||x||
                nc.scalar.activation(nrm4[:], ssq4[:], Act.Sqrt, scale=0.01)
                nc.vector.reciprocal(xinv[:, g * SGT:(g + 1) * SGT], nrm4[:])

            # proto processing is staged (norms / scale / transpose+cast
            # emitted as separate waves across subgroups) so the per-subgroup
            # ACT->DVE->PE->cast chains pipeline instead of serializing in
            # the engine FIFOs.
            pinv = {}

            def p_norms(g):
                ssq4 = norms4(psg[g], "p")
                nrm4 = pool.tile([128, SGT], f32, tag=f"nrm4p{g % 2}")
                nc.scalar.activation(nrm4[:], ssq4[:], Act.Sqrt)
                inv4 = pool.tile([128, SGT], f32, tag=f"inv4p{g}")
                nc.vector.reciprocal(inv4[:], nrm4[:])
                nc.vector.tensor_scalar_mul(inv4[:], inv4[:], sc_b[:])
                pinv[g] = inv4

            def p_scale(g):
                for t in range(SGT):
                    src = psg[g][:, t * D:(t + 1) * D]
                    nc.vector.tensor_scalar_mul(src, src, pinv[g][:, t:t + 1])

            def p_transcast(g):
                transpose_cast(psg[g], g, pt_r)

            def p_pair(g0):
                # pipelined emission over subgroups g0, g0+1; both scales
                # before both transcasts so the second subgroup's scale isn't
                # serialized behind the first subgroup's casts on DVE
                p_norms(g0)
                p_norms(g0 + 1)
                p_scale(g0)
                p_scale(g0 + 1)
                p_transcast(g0)
                p_transcast(g0 + 1)

            # ---- main matmul + scaling drain ----
            # phase h covers n-blocks {2h, 2h+1} (proto subgroups 2h, 2h+1);
            # per b-tile i: 4 fp16 MMs (k-outer) into one 2-bank PSUM tile,
            # one 1024-wide drain that also applies 10/||x_b||, one 512KB
            # output DMA (128 rows x 4KB).
            def mm(h, i):
                oq = outp.tile([128, 1024], f32, tag="oq")
                ps = psum_m.tile([128, 1024], f32, tag="mm")
                for k in range(NK):
                    for nn_ in range(2):
                        n = 2 * h + nn_
                        nc.tensor.matmul(
                            ps[:, nn_ * 512:(nn_ + 1) * 512],
                            xt_r[:, i, k, :],
                            pt_r[:, 4 * n:4 * n + 4, k, :],
                            start=(k == 0), stop=(k == NK - 1))
                inv = xinv[:, i:i + 1]
                # phase-dependent engine split: phase 0's window is crowded
                # with x1..x3/p2..p3 processing on ACT, later phases are not
                act_mod = (4, 3, 3, 2)[h]
                if i % act_mod == 0:
                    nc.scalar.activation(oq[:], ps[:], Act.Copy, scale=inv)
                else:
                    nc.vector.tensor_scalar_mul(oq[:], ps[:], inv)
                nc.sync.dma_start(
                    out_d[i * 128:(i + 1) * 128,
                          h * 1024:(h + 1) * 1024], oq[:])

            # emission interleaved with processing so engine FIFOs don't
            # head-of-line block early drains, and late processing is spread
            # thin across the mm stream so it never stalls the output DMAs.
            # Constraints: px_cast(g)/px_norms(g) before mm(*, 4g);
            # p_scale/p_transcast(2h..2h+1) before mm(h, 0).
            process_sched = {
                0: {1: [lambda: px_norms(1)],
                    2: [lambda: px_cast(1)],
                    4: [lambda: p_norms(2)],
                    5: [lambda: px_norms(2)],
                    6: [lambda: px_cast(2)],
                    7: [lambda: p_norms(3)],
                    8: [lambda: p_scale(2)],
                    9: [lambda: px_norms(3)],
                    10: [lambda: p_transcast(2)],
                    11: [lambda: px_cast(3)],
                    12: [lambda: p_scale(3)],
                    13: [lambda: p_transcast(3)]},
                1: {1: [lambda: p_norms(4)],
                    3: [lambda: p_norms(5)],
                    5: [lambda: p_scale(4)],
                    7: [lambda: p_transcast(4)],
                    9: [lambda: p_scale(5)],
                    11: [lambda: p_transcast(5)]},
                2: {1: [lambda: p_norms(6)],
                    3: [lambda: p_norms(7)],
                    5: [lambda: p_scale(6)],
                    7: [lambda: p_transcast(6)],
                    9: [lambda: p_scale(7)],
                    11: [lambda: p_transcast(7)]},
                3: {},
            }
            px_norms(0)
            px_cast(0)
            p_pair(0)
            for h in range(NPH):
                for i in range(NB):
                    mm(h, i)
                    for fn in process_sched[h].get(i, []):
                        fn()

    nc.compile()
    return nc


def _get_compiled():
    global _compiled
    if _compiled is None:
        _compiled = _build()
    return _compiled


def kernel(inputs, proto, scalar, _trace=False, **_tr_kw):
    from concourse.bass_utils import run_bass_kernel_spmd

    nc = _get_compiled()
    inputs = np.ascontiguousarray(inputs, dtype=np.float32)
    proto = np.ascontiguousarray(proto, dtype=np.float32)
    sc = np.asarray(scalar, dtype=np.float32).reshape(1, 1)
    ident = np.eye(128, dtype=np.float32)

    in_maps = []
    for c in range(NCORES):
        in_maps.append({
            "x": inputs[c * BS:(c + 1) * BS],
            "proto": proto,
            "scalar": sc,
            "identity": ident,
        })
    res = run_bass_kernel_spmd(nc, in_maps, core_ids=list(range(NCORES)),
                               trace=_trace, **_tr_kw)
    out = np.concatenate([res.results[c]["out"] for c in range(NCORES)], axis=0)
    if _trace:
        kernel.last_results = res
    return out
